# revision 13
# baseline (speedup 1.0000x reference)
"""Trainium2 Bass kernel for nn_AggregationEncoder (gnn_message_passing).

Reference computation:
    adj[g, m] = 1 where an edge (g, m) exists (set semantics)
    norm[m]   = max(sum_g adj[g, m], 1)
    out[b, m, d] = sum_g adj[g, m] / norm[m] * x[b, g, d]

Structural facts hardcoded from the problem spec:
  - x: [B=2, G=40962, D=512] float32; edge_index: [E=122880, 2] int64,
    BOTH columns in [0, 2562) -> contraction only needs x[:, :2562, :].
  - M = 2562 mesh nodes.

Sharding (8 cores): 2 batches x 4 mesh-column chunks of W=642 columns.
Host work is sharding/layout only: dedup the edge set, lay it out as a
dense 0/1 bf16 adjacency chunk the device DMAs directly, pre-cast x to
bf16, precompute per-column reciprocal degrees (pure function of
edge_index).

Ragged-dimension trick (2562 = 4*642 - 6, 642 = 5*128 + 2): columns
0..639 come from the main 21-ktile matmul stream over senders 0..2559
(+ leftover senders 2560/2561 whose edges live in k-tile 20 rows 0/1).
Columns 640/641 come from ONE extra matmul contracting only k-tile 20:
the host gathers those receivers' sender rows into x pad rows
2562..2687 and marks them in adjacency columns 640/641.

Measured regime: the kernel is DMA-wire-bound (~235 GB/s/core aggregate)
with the PE stream (106 matmuls x 216ns) fitting inside the wire time,
so inputs use bf16/partition-blocked chunk tensors (per-partition
contiguous, sequential descriptors), outputs ship as bf16 (host upcasts;
error stays ~4x under the 2e-2 gate), and the first chunks are tiny so
the stream starts during wire warm-up. Warm-up matmuls hold the PE
p-state through the pre-data window (a >1us idle gap resets the clock).
"""

import numpy as np
import ml_dtypes

B = 2
G = 40962
D = 512
M = 2562            # mesh nodes
SEN = 2562          # senders (edge values < 2562)
GP = 2688           # padded sender rows = 21*128
KT = GP // 128      # 21 k-tiles
NQ = 4              # mesh-column chunks
W = 642             # mesh columns per chunk (5*128 + 2)
WMAIN = 640         # columns via the main 21-ktile stream
NMT = 5             # full 128-col m-tiles
PAD0 = 2562         # first gather-pad row
NPAD = GP - PAD0    # 126 gather slots
N_CORES = 8

# k-tile chunking for input DMAs: tiny head (stream starts during HBM
# warm-up), big tail (amortize ~0.7us doorbells).
CHUNKS = [(0, 1), (1, 3), (3, 9), (9, 15), (15, 21)]

_NC_CACHE = None


def _build_bass():
    import concourse.bacc as bacc
    import concourse.mybir as mybir
    import concourse.tile as tile

    dt = mybir.dt
    nc = bacc.Bacc("TRN2", target_bir_lowering=False, debug=False,
                   num_devices=N_CORES)

    xhs = [nc.dram_tensor(f"xh{i}", [128, k1 - k0, D], dt.bfloat16,
                          kind="ExternalInput")
           for i, (k0, k1) in enumerate(CHUNKS)]
    adjs = [nc.dram_tensor(f"adj{i}", [128, k1 - k0, W], dt.bfloat16,
                           kind="ExternalInput")
            for i, (k0, k1) in enumerate(CHUNKS)]
    recip = nc.dram_tensor("recip", [128, 6], dt.float32,
                           kind="ExternalInput")
    out = nc.dram_tensor("out", [W, D], dt.bfloat16, kind="ExternalOutput")

    with tile.TileContext(nc) as tc:
        with (
            tc.tile_pool(name="sbuf", bufs=1) as sb,
            tc.tile_pool(name="outb", bufs=2) as outb,
            tc.tile_pool(name="psum", bufs=1, space="PSUM") as ps,
        ):
            a_sb = sb.tile([128, KT, W], dt.bfloat16)
            x_sb = sb.tile([128, KT, D], dt.bfloat16)
            recip_sb = sb.tile([128, 6], dt.float32)
            for i, (k0, k1) in enumerate(CHUNKS):
                nc.sync.dma_start(out=x_sb[:, k0:k1, :], in_=xhs[i][:])
                nc.scalar.dma_start(out=a_sb[:, k0:k1, :], in_=adjs[i][:])
                if i == 1:
                    nc.sync.dma_start(recip_sb[:], recip[:])

            psums = [ps.tile([128, D], dt.float32, tag=f"ps{mt}",
                             name=f"psum{mt}")
                     for mt in range(NMT)]
            pst = ps.tile([2, D], dt.float32, tag="pst", name="psum_tiny")

            # Warm-up matmuls keep the PE p-state ramping through the
            # pre-data window. Two double as micro-probes: fp8 moving
            # operand (x bf16 weights) and fp8xfp8, to read their row
            # rates off the trace.
            warm_src = sb.tile([128, D], dt.bfloat16)
            nc.vector.memset(warm_src[:], 1.0)
            warm8 = sb.tile([128, D], dt.float8e4)
            nc.vector.memset(warm8[:], 1.0)
            warm = ps.tile([32, D], dt.float32, tag="warm", name="warm")
            nc.tensor.matmul(warm[:, :], lhsT=warm_src[:, 0:32],
                             rhs=warm_src[:], start=True, stop=True)
            nc.tensor.matmul(warm[:, :], lhsT=warm_src[:, 0:32],
                             rhs=warm8[:], start=True, stop=True)
            nc.tensor.matmul(warm[:, :], lhsT=warm8[:, 0:32],
                             rhs=warm8[:], start=True, stop=True)
            for _ in range(2):
                nc.tensor.matmul(warm[:, :], lhsT=warm_src[:, 0:32],
                                 rhs=warm_src[:], start=True, stop=True)

            def mm(mt, kt):
                nc.tensor.matmul(
                    psums[mt][:, :],
                    lhsT=a_sb[:, kt, mt * 128:(mt + 1) * 128],
                    rhs=x_sb[:, kt, :],
                    start=(kt == 0),
                    stop=(kt == KT - 1),
                )

            for t in range(6):               # kts 0..17
                for mt in range(NMT):
                    for kt in (3 * t, 3 * t + 1, 3 * t + 2):
                        mm(mt, kt)

            def drain(mt):
                o_sb = outb.tile([128, D], dt.bfloat16, tag=f"osb{mt % 2}",
                                 name=f"osb{mt}")
                if mt % 2 == 0:
                    nc.vector.tensor_scalar_mul(
                        o_sb[:], psums[mt][:, 0:D], recip_sb[:, mt:mt + 1])
                    nc.sync.dma_start(out[mt * 128:(mt + 1) * 128, :],
                                      o_sb[:])
                else:
                    nc.scalar.activation(
                        o_sb[:], psums[mt][:, 0:D],
                        mybir.ActivationFunctionType.Copy,
                        scale=recip_sb[:, mt:mt + 1])
                    nc.scalar.dma_start(out[mt * 128:(mt + 1) * 128, :],
                                        o_sb[:])

            for mt in range(NMT - 1):
                for kt in (18, 19, 20):
                    mm(mt, kt)
                drain(mt)

            # tiny columns 640/641 (contract only k-tile 20) BEFORE mt4's
            # final group so its Vector+SP drain hides under mt4's
            # matmuls; only mt4's Scalar chain is exposed at the end.
            nc.tensor.matmul(
                pst[:, :],
                lhsT=a_sb[:, KT - 1, WMAIN:W],
                rhs=x_sb[:, KT - 1, :],
                start=True,
                stop=True,
            )
            ot = outb.tile([2, D], dt.bfloat16, tag="osbt", name="osb_t")
            nc.vector.tensor_scalar_mul(ot[:], pst[:, 0:D],
                                        recip_sb[0:2, 5:6])
            nc.sync.dma_start(out[WMAIN:W, :], ot[:])

            for kt in (18, 19, 20):
                mm(NMT - 1, kt)
            o4 = outb.tile([128, D], dt.bfloat16, tag="osb1", name="osb4")
            nc.scalar.activation(o4[:], psums[NMT - 1][:, 0:D],
                                 mybir.ActivationFunctionType.Copy,
                                 scale=recip_sb[:, NMT - 1:NMT])
            nc.scalar.dma_start(out[(NMT - 1) * 128:NMT * 128, :], o4[:])

    nc.finalize()
    return nc


def _get_nc():
    global _NC_CACHE
    if _NC_CACHE is None:
        _NC_CACHE = _build_bass()
    return _NC_CACHE


def _blocks(a):
    """[GP, F] row-major -> per-chunk [128, nk, F] partition-blocked."""
    outl = []
    for k0, k1 in CHUNKS:
        blk = a[k0 * 128:k1 * 128]
        outl.append(np.ascontiguousarray(
            blk.reshape(k1 - k0, 128, a.shape[1]).transpose(1, 0, 2)))
    return outl


def _host_shard(grid_node_features, edge_index):
    """Dedup edges and lay them out as per-chunk dense adjacency + padded
    bf16 x + reciprocal degrees. Returns per-core input maps."""
    x = np.asarray(grid_node_features)
    e = np.asarray(edge_index)
    g = e[:, 0].astype(np.int64)
    m = e[:, 1].astype(np.int64)
    key = np.unique(g * M + m)           # set semantics
    g = key // M
    m = key % M
    deg = np.bincount(m, minlength=M).astype(np.float64)
    rec_full = (1.0 / np.maximum(deg, 1.0)).astype(np.float32)

    ONE16 = np.uint16(0x3F80)            # bf16 1.0

    adj_blocks = []
    recs = []
    glists = []
    for q in range(NQ):
        lo = q * W
        sel = (m >= lo) & (m < lo + W)
        gq = g[sel]
        mq = m[sel] - lo
        av = np.zeros((GP, W), np.uint16)
        main = mq < WMAIN
        av[gq[main], mq[main]] = ONE16
        glist = []
        for r in (WMAIN, WMAIN + 1):
            if lo + r >= M:
                continue
            snd = np.sort(gq[mq == r])
            av[snd[snd >= 2560], r] = ONE16
            for s in snd[snd < 2560]:
                av[PAD0 + len(glist), r] = ONE16
                glist.append(s)
        if len(glist) > NPAD:
            raise ValueError(f"gather overflow: {len(glist)} > {NPAD}")
        adj_blocks.append([b.view(ml_dtypes.bfloat16)
                           for b in _blocks(av)])
        glists.append(np.asarray(glist, np.int64))

        rv = np.zeros((128, 6), np.float32)
        for mt in range(NMT):
            c0 = lo + mt * 128
            n = min(128, max(M - c0, 0))
            if n > 0:
                rv[:n, mt] = rec_full[c0:c0 + n]
        for j in range(2):
            if lo + WMAIN + j < M:
                rv[j, 5] = rec_full[lo + WMAIN + j]
        recs.append(rv)

    in_maps = [None] * N_CORES
    for b in range(B):
        xb = x[b, :SEN, :].astype(ml_dtypes.bfloat16)
        for q in range(NQ):
            xp = np.zeros((GP, D), ml_dtypes.bfloat16)
            xp[:SEN] = xb
            gl = glists[q]
            if gl.size:
                xp[PAD0:PAD0 + gl.size] = xb[gl]
            im = {"recip": recs[q]}
            for i, blk in enumerate(_blocks(xp)):
                im[f"xh{i}"] = blk
            for i, blk in enumerate(adj_blocks[q]):
                im[f"adj{i}"] = blk
            in_maps[b * NQ + q] = im
    return in_maps


def kernel(grid_node_features, edge_index):
    from concourse.bass_utils import run_bass_kernel_spmd

    nc = _get_nc()
    in_maps = _host_shard(grid_node_features, edge_index)
    res = run_bass_kernel_spmd(nc, in_maps, core_ids=list(range(N_CORES)))

    out = np.empty((B, M, D), dtype=np.float32)
    for c in range(N_CORES):
        b, q = divmod(c, NQ)
        lo = q * W
        cq = min(W, M - lo)
        out[b, lo:lo + cq, :] = res.results[c]["out"][:cq, :].astype(
            np.float32)
    return out


# revision 14
# speedup vs baseline: 1.1411x; 1.1411x over previous
"""Trainium2 Bass kernel for nn_AggregationEncoder (gnn_message_passing).

Reference computation:
    adj[g, m] = 1 where an edge (g, m) exists (set semantics)
    norm[m]   = max(sum_g adj[g, m], 1)
    out[b, m, d] = sum_g adj[g, m] / norm[m] * x[b, g, d]

Structural facts hardcoded from the problem spec:
  - x: [B=2, G=40962, D=512] float32; edge_index: [E=122880, 2] int64,
    BOTH columns in [0, 2562) -> contraction only needs x[:, :2562, :].
  - M = 2562 mesh nodes.

Sharding (8 cores): 2 batches x 4 mesh-column chunks of W=642 columns.
Host work is sharding/layout only: dedup the edge set, lay it out as a
dense 0/1 fp8 adjacency chunk the device DMAs directly, pre-cast x to
bf16, precompute per-column reciprocal degrees (pure function of
edge_index).

Orientation: the kernel computes out^T[d, m] with x tiles as the
STATIONARY operand (bf16 weights) and the adjacency as the MOVING
operand in fp8e4m3 (measured full-rate as moving; fp8 *weights* run
~20% slow). This halves adjacency bytes — the kernel is DMA-wire-bound
(~235 GB/s/core aggregate) — and the ragged mesh columns 640/641 ride
along in the 130-wide column group (their senders are host-gathered
into x pad rows 2562..2687, adjacency rows restricted to k-tile 20).
Normalization is a DVE multiply against a host-replicated bf16
reciprocal row; outputs ship as bf16 transposed and the host
reassembles (error stays ~3x under the 2e-2 gate).

Per kt (21 of them): 4 d-tiles x (512+130 moving rows) = 8 matmuls into
8 PSUM banks, accumulated across all kts; final k-group is d-tile-major
so drains (DVE normalize + output DMA on alternating rings) overlap the
stream tail. Warm-up matmuls hold the PE p-state through the pre-data
window (a >1us idle gap resets the clock and costs a re-ramp).
"""

import numpy as np
import ml_dtypes

B = 2
G = 40962
D = 512
M = 2562            # mesh nodes
SEN = 2562          # senders (edge values < 2562)
GP = 2688           # padded sender rows = 21*128
KT = GP // 128      # 21 k-tiles
NQ = 4              # mesh-column chunks
W = 642             # mesh columns per chunk
WA = 512            # first moving column group
WB = W - WA         # second moving column group (incl. ragged cols)
NDT = 4             # d-tiles of 128
PAD0 = 2562         # first gather-pad row
NPAD = GP - PAD0    # 126 gather slots
N_CORES = 8

SC = 7              # input DMA chunks of PL=3 k-tiles
PL = 3

_NC_CACHE = None


def _build_bass():
    import concourse.bacc as bacc
    import concourse.mybir as mybir
    import concourse.tile as tile

    dt = mybir.dt
    nc = bacc.Bacc("TRN2", target_bir_lowering=False, debug=False,
                   num_devices=N_CORES)

    # DRAM chunk blocks: partition-major inside each 3-ktile block so
    # every DMA descriptor is one partition's contiguous run and the
    # descriptor stream is sequential in DRAM.
    xh = nc.dram_tensor("xh", [SC, 128, PL, D], dt.bfloat16,
                        kind="ExternalInput")
    adj = nc.dram_tensor("adj", [SC, 128, PL, W], dt.float8e4,
                         kind="ExternalInput")
    rb = nc.dram_tensor("rb", [128, W], dt.bfloat16, kind="ExternalInput")
    out = nc.dram_tensor("out", [D, W], dt.bfloat16, kind="ExternalOutput")

    with tile.TileContext(nc) as tc:
        with (
            tc.tile_pool(name="sbuf", bufs=1) as sb,
            tc.tile_pool(name="outb", bufs=2) as outb,
            tc.tile_pool(name="psum", bufs=1, space="PSUM") as ps,
        ):
            a_sb = sb.tile([128, KT, W], dt.float8e4)
            x_sb = sb.tile([128, KT, D], dt.bfloat16)
            rb_sb = sb.tile([128, W], dt.bfloat16)
            for s in range(SC):
                nc.sync.dma_start(out=x_sb[:, s * PL:(s + 1) * PL, :],
                                  in_=xh[s])
                nc.scalar.dma_start(out=a_sb[:, s * PL:(s + 1) * PL, :],
                                    in_=adj[s])
                if s == 1:
                    nc.scalar.dma_start(rb_sb[:], rb[:])

            pA = [ps.tile([128, WA], dt.float32, tag=f"pa{i}",
                          name=f"pa{i}") for i in range(NDT)]
            pB = [ps.tile([128, WB], dt.float32, tag=f"pb{i}",
                          name=f"pb{i}") for i in range(NDT)]

            # Warm-ups (PE p-state) — same dtype mix as the real stream;
            # they share pA[0], which kt0's start=True reset clears.
            warm_src = sb.tile([128, D], dt.bfloat16)
            nc.vector.memset(warm_src[:], 1.0)
            warm8 = sb.tile([128, WA], dt.float8e4)
            nc.vector.memset(warm8[:], 1.0)
            for _ in range(5):
                nc.tensor.matmul(pA[0][0:32, :], lhsT=warm_src[:, 0:32],
                                 rhs=warm8[:], start=True, stop=True)

            def mm(dtile, kt):
                lhs = x_sb[:, kt, dtile * 128:(dtile + 1) * 128]
                nc.tensor.matmul(pA[dtile][:, :], lhsT=lhs,
                                 rhs=a_sb[:, kt, 0:WA],
                                 start=(kt == 0), stop=(kt == KT - 1))
                nc.tensor.matmul(pB[dtile][:, :], lhsT=lhs,
                                 rhs=a_sb[:, kt, WA:W],
                                 start=(kt == 0), stop=(kt == KT - 1))

            for t in range(SC - 1):          # kts 0..17
                for kt in (3 * t, 3 * t + 1, 3 * t + 2):
                    for dtile in range(NDT):
                        mm(dtile, kt)

            # Final k-group d-tile-major: each d-tile finishes, then its
            # normalize+store overlaps the remaining matmuls.
            for dtile in range(NDT):
                for kt in (18, 19, 20):
                    mm(dtile, kt)
                o_sb = outb.tile([128, W], dt.bfloat16,
                                 tag=f"osb{dtile % 2}", name=f"osb{dtile}")
                nc.vector.tensor_mul(o_sb[:, 0:WA], pA[dtile][:, :],
                                     rb_sb[:, 0:WA])
                nc.vector.tensor_mul(o_sb[:, WA:W], pB[dtile][:, :],
                                     rb_sb[:, WA:W])
                if dtile % 2 == 0:
                    nc.sync.dma_start(
                        out[dtile * 128:(dtile + 1) * 128, :], o_sb[:])
                else:
                    nc.scalar.dma_start(
                        out[dtile * 128:(dtile + 1) * 128, :], o_sb[:])

    nc.finalize()
    return nc


def _get_nc():
    global _NC_CACHE
    if _NC_CACHE is None:
        _NC_CACHE = _build_bass()
    return _NC_CACHE


def _pm(a):
    """[GP, F] row-major -> [SC, 128, PL, F] 3-plane partition-minor."""
    return np.ascontiguousarray(
        a.reshape(SC, PL, 128, a.shape[1]).transpose(0, 2, 1, 3))


def _host_shard(grid_node_features, edge_index):
    """Dedup edges and lay them out as per-chunk dense adjacency + padded
    bf16 x + replicated reciprocal degrees. Returns per-core inputs."""
    x = np.asarray(grid_node_features)
    e = np.asarray(edge_index)
    g = e[:, 0].astype(np.int64)
    m = e[:, 1].astype(np.int64)
    key = np.unique(g * M + m)           # set semantics
    g = key // M
    m = key % M
    deg = np.bincount(m, minlength=M).astype(np.float64)
    rec_full = (1.0 / np.maximum(deg, 1.0)).astype(np.float32)

    ONE8 = np.uint8(0x38)                # fp8 e4m3 1.0

    adjs = []
    rbs = []
    glists = []
    for q in range(NQ):
        lo = q * W
        sel = (m >= lo) & (m < lo + W)
        gq = g[sel]
        mq = m[sel] - lo
        av = np.zeros((GP, W), np.uint8)
        main = mq < W - 2
        av[gq[main], mq[main]] = ONE8
        # ragged columns 640/641: senders >= 2560 sit in k-tile 20
        # already; senders < 2560 are gathered into pad rows.
        glist = []
        for r in (W - 2, W - 1):
            if lo + r >= M:
                continue
            snd = np.sort(gq[mq == r])
            av[snd[snd >= 2560], r] = ONE8
            for s in snd[snd < 2560]:
                av[PAD0 + len(glist), r] = ONE8
                glist.append(s)
        if len(glist) > NPAD:
            raise ValueError(f"gather overflow: {len(glist)} > {NPAD}")
        adjs.append(_pm(av).view(ml_dtypes.float8_e4m3))
        glists.append(np.asarray(glist, np.int64))

        rv = np.zeros(W, np.float32)
        n = min(W, M - lo)
        rv[:n] = rec_full[lo:lo + n]
        rbs.append(np.ascontiguousarray(
            np.broadcast_to(rv.astype(ml_dtypes.bfloat16), (128, W))))

    in_maps = [None] * N_CORES
    for b in range(B):
        xb = x[b, :SEN, :].astype(ml_dtypes.bfloat16)
        for q in range(NQ):
            xp = np.zeros((GP, D), ml_dtypes.bfloat16)
            xp[:SEN] = xb
            gl = glists[q]
            if gl.size:
                xp[PAD0:PAD0 + gl.size] = xb[gl]
            in_maps[b * NQ + q] = {"xh": _pm(xp), "adj": adjs[q],
                                   "rb": rbs[q]}
    return in_maps


def kernel(grid_node_features, edge_index):
    from concourse.bass_utils import run_bass_kernel_spmd

    nc = _get_nc()
    in_maps = _host_shard(grid_node_features, edge_index)
    res = run_bass_kernel_spmd(nc, in_maps, core_ids=list(range(N_CORES)))

    out = np.empty((B, M, D), dtype=np.float32)
    for c in range(N_CORES):
        b, q = divmod(c, NQ)
        lo = q * W
        cq = min(W, M - lo)
        out[b, lo:lo + cq, :] = res.results[c]["out"][:, :cq].T.astype(
            np.float32)
    return out
